# revision 1
# baseline (speedup 1.0000x reference)
"""Distributed GAT (AnomalyDAE encoder) kernel for 8 TRN2 NeuronCores.

Reference computation:
    h = leaky_relu(x @ W_dense.T + b_dense, 0.01)          # [N, 128]
    g = h @ W_gat.T                                        # [N, 64]
    a_src = g @ att_src ; a_dst = g @ att_dst              # [N]
    with self-loops appended, per edge (s -> d):
        e = leaky_relu(a_src[s] + a_dst[d], 0.2)
        alpha = segment_softmax(e, by d)
    out[d] = sum_e alpha_e * g[s_e] + b_gat                # [N, 64]

Sharding: nodes split contiguously across 8 cores (6250 each); edges
partitioned by destination core. Each core's local nodes are sorted by
in-degree on the host so 128-node tiles have near-uniform degree; per-tile
edge lists are padded to the tile max degree.

Device pipeline per core:
  node phase: per 128-node tile, matmuls against host-pretransposed x
    -> hT -> lrelu -> gT -> a_src/a_dst matvec -> PE-transpose -> table rows
    [g bf16(64) | a_src f32 | a_dst f32 | pad] (256 B) in a DRAM shard.
  AllGather shards -> full table [R*NL_pad, 128] bf16-typed rows.
  edge phase: per 128-dst tile, two gpsimd dma_gather calls (table halves A/B
    -- int16 gather indices limit a call's index space to <32768 rows), then
    exp(LRelu(s)) = max(e^s, e^{0.2 s}) on ACT, tensor_tensor_reduce for the
    unnormalized weights + denominator, fused normalize+weight on DVE, reduce
    over neighbors, + b_gat, write out rows.

Softmax is computed without the segment-max shift (logits are O(1); result
identical in exact arithmetic, and the two half-table partial sums compose
exactly). Padded slots gather a sentinel row with a_src = -80 so their
weight underflows to 0; pad destinations use a neutral zero row.
"""

import numpy as np
import ml_dtypes

bf16 = ml_dtypes.bfloat16

R = 8            # cores
HALF = R // 2
P = 128          # partitions / tile size
W_ROW = 128      # table row width in bf16 elems (256 B rows for dma_gather)
A_SRC_F32 = 32   # f32 column of a_src within a row (byte offset 128)
A_DST_F32 = 33
SENT_VAL = -80.0
DCALL = 8      # max gather rows per dma_gather call (1024-descriptor ucode ring)


class Cfg:
    def __init__(self, N, E, IN=512, EMB=128, OUT=64):
        assert N % R == 0
        self.N, self.E, self.IN, self.EMB, self.OUT = N, E, IN, EMB, OUT
        self.NL = N // R
        nlp = ((self.NL + 2 + P - 1) // P) * P   # >= 2 spare rows per shard
        self.NL_pad = nlp
        self.TILES = nlp // P
        self.NTAB = nlp * R
        self.NHALF = nlp * HALF                  # rows per table half
        assert self.NHALF < 32768, "dma_gather int16 index limit"


CFG_REAL = Cfg(N=50000, E=1600000)


# --------------------------------------------------------------------------
# host-side preprocessing
# --------------------------------------------------------------------------

def _wrap_idx(lin):
    """dma_gather index layout: linear i -> [i % 16, i // 16], replicated
    across the 8 Q7 core groups -> [128, len/16] int16."""
    assert len(lin) % 16 == 0
    w = lin.reshape(-1, 16).T.astype(np.int16)
    return np.tile(w, (8, 1))


def _prepare(cfg, x, edge_index, W_dense, b_dense, W_gat, att_src, att_dst,
             b_gat):
    N, NL, NL_pad, TILES = cfg.N, cfg.NL, cfg.NL_pad, cfg.TILES
    src = edge_index[0].astype(np.int64)
    dst = edge_index[1].astype(np.int64)
    loops = np.arange(N, dtype=np.int64)
    src = np.concatenate([src, loops])
    dst = np.concatenate([dst, loops])

    deg = np.bincount(dst, minlength=N)

    pos_of = np.empty(N, dtype=np.int64)
    orders = []
    for r in range(R):
        dloc = deg[r * NL:(r + 1) * NL]
        order = np.argsort(-dloc, kind="stable")
        orders.append(order)
        pos_of[r * NL + order] = np.arange(NL)

    core_of_node = np.arange(N) // NL
    gid_of = core_of_node * NL_pad + pos_of          # global table row
    in_b = gid_of >= cfg.NHALF                       # source in half B

    neutral = NL_pad - 2     # local row in the first shard of each half
    sent = NL_pad - 1

    # per-(core,position,half) degrees
    degh = np.zeros((2, R * NL_pad), dtype=np.int64)
    dst_key = core_of_node[dst] * NL_pad + pos_of[dst]
    np.add.at(degh[0], dst_key[~in_b[src]], 1)
    np.add.at(degh[1], dst_key[in_b[src]], 1)
    D_lists = []
    for h in range(2):
        dt_ = degh[h].reshape(R, TILES, P).max(axis=(0, 2))
        D_lists.append(np.maximum(dt_, 1).astype(np.int64))

    # sort edges by (dst position, half, src gid)
    ekey = dst_key * 2 + in_b[src]
    eorder = np.lexsort((gid_of[src], ekey))
    src_s = src[eorder]
    dst_key_s = dst_key[eorder]
    hb_s = in_b[src_s]
    # slot index within the (dst, half) group
    grp = dst_key_s * 2 + hb_s
    starts = np.zeros(2 * R * NL_pad + 1, dtype=np.int64)
    np.add.at(starts, grp + 1, 1)
    starts = np.cumsum(starts)
    k_of = np.arange(len(src_s)) - starts[grp]

    # offs[h] : int32 [R, NL_pad, Dmax_h] filled with sentinel
    offs = []
    for h in range(2):
        Dmax = int(D_lists[h].max())
        o = np.full((R, NL_pad, Dmax), sent, dtype=np.int32)
        o[:, NL:, :] = neutral
        m = hb_s == bool(h)
        gl = gid_of[src_s[m]] - (cfg.NHALF if h else 0)
        o[dst_key_s[m] // NL_pad, dst_key_s[m] % NL_pad, k_of[m]] = gl
        offs.append(o)

    in_maps = []
    wdT = np.ascontiguousarray(W_dense.T)            # [IN, EMB]
    wdT_packed = np.concatenate(
        [wdT[k * P:(k + 1) * P, :] for k in range(cfg.IN // P)], axis=1)
    att = np.stack([att_src, att_dst], axis=1)       # [OUT, 2]
    KC = cfg.IN // P
    for r in range(R):
        xp = np.zeros((NL_pad, cfg.IN), dtype=np.float32)
        xp[:NL] = x[r * NL + orders[r]]
        # transposed, tiled: block (t, k) = xp[tP:(t+1)P, kP:(k+1)P].T
        xT = np.empty((P, TILES * KC * P), dtype=bf16)
        for t in range(TILES):
            blk = xp[t * P:(t + 1) * P, :].T.astype(bf16)   # [IN, P]
            xT[:, (t * KC) * P:(t + 1) * KC * P] = \
                blk.reshape(KC, P, P).transpose(1, 0, 2).reshape(P, KC * P)
        offs_w = []
        for h in range(2):
            cols = []
            for t in range(TILES):
                Dt = int(D_lists[h][t])
                lin = offs[h][r, t * P:(t + 1) * P, :Dt]     # [P, Dt]
                # linear slot i = k*128 + p
                cols.append(_wrap_idx(lin.T.reshape(-1)))
            offs_w.append(np.ascontiguousarray(np.concatenate(cols, axis=1)))
        in_maps.append({
            "xT": xT,
            "wdT": wdT_packed.astype(bf16),
            "bd": b_dense.reshape(cfg.EMB, 1).astype(np.float32),
            "wgT": np.ascontiguousarray(W_gat.T).astype(bf16),
            "att": att.astype(bf16),
            "bgat": b_gat.reshape(cfg.OUT, 1).astype(np.float32),
            "offsA": offs_w[0],
            "offsB": offs_w[1],
        })
    return in_maps, orders, D_lists


def _assemble(cfg, results, orders):
    out = np.empty((cfg.N, cfg.OUT), dtype=np.float32)
    for r in range(R):
        o = results[r]["out"][:cfg.NL]
        out[r * cfg.NL + orders[r]] = o
    return out


# --------------------------------------------------------------------------
# device graph
# --------------------------------------------------------------------------

def _build_graph(cfg, D_lists):
    import concourse.bass as bass
    import concourse.bacc as bacc
    import concourse.mybir as mybir
    import concourse.tile as tile
    from concourse.masks import make_identity

    IN, EMB, OUT = cfg.IN, cfg.EMB, cfg.OUT
    KC = IN // P
    TILES, NL_pad, NTAB, NHALF = cfg.TILES, cfg.NL_pad, cfg.NTAB, cfg.NHALF
    TOT = [int(d.sum()) for d in D_lists]
    fp32 = mybir.dt.float32
    b16 = mybir.dt.bfloat16
    i16 = mybir.dt.int16

    nc = bacc.Bacc(None, target_bir_lowering=False, debug=False, num_devices=R,
                   num_swdge_queues=4)

    xT = nc.dram_tensor("xT", [P, TILES * KC * P], b16, kind="ExternalInput")
    wdT = nc.dram_tensor("wdT", [P, KC * EMB], b16, kind="ExternalInput")
    bd = nc.dram_tensor("bd", [EMB, 1], fp32, kind="ExternalInput")
    wgT = nc.dram_tensor("wgT", [EMB, OUT], b16, kind="ExternalInput")
    att = nc.dram_tensor("att", [OUT, 2], b16, kind="ExternalInput")
    bgat = nc.dram_tensor("bgat", [OUT, 1], fp32, kind="ExternalInput")
    offs_ext = [
        nc.dram_tensor("offsA", [P, 8 * TOT[0]], i16, kind="ExternalInput"),
        nc.dram_tensor("offsB", [P, 8 * TOT[1]], i16, kind="ExternalInput"),
    ]
    out = nc.dram_tensor("out", [NL_pad, OUT], fp32, kind="ExternalOutput")

    with tile.TileContext(nc) as tc:
        with (
            tc.tile_pool(name="dram", bufs=1, space="DRAM") as dram,
            tc.tile_pool(name="const", bufs=1) as cst,
        ):
            shard = dram.tile([NL_pad, W_ROW], b16)
            full = dram.tile([NTAB, W_ROW], b16, addr_space="Shared")

            identb = cst.tile([P, P], b16)
            make_identity(nc, identb[:])
            identf = cst.tile([P, P], fp32)
            make_identity(nc, identf[:])

            wdTs = cst.tile([P, KC * EMB], b16)
            nc.sync.dma_start(out=wdTs[:], in_=wdT[:, :])
            bds = cst.tile([EMB, 1], fp32)
            nc.sync.dma_start(out=bds[:], in_=bd[:, :])
            wgTs = cst.tile([EMB, OUT], b16)
            nc.sync.dma_start(out=wgTs[:], in_=wgT[:, :])
            atts = cst.tile([OUT, 2], b16)
            nc.sync.dma_start(out=atts[:], in_=att[:, :])
            bgs = cst.tile([OUT, 1], fp32)
            nc.sync.dma_start(out=bgs[:], in_=bgat[:, :])
            sentc = cst.tile([1, 1], fp32)
            nc.gpsimd.memset(sentc[:], SENT_VAL)
            adst_all = cst.tile([P, TILES], fp32)

            shard_f32 = shard[:].bitcast(fp32)       # [NL_pad, 64]

            # ---------------- node phase ----------------
            with (
                tc.tile_pool(name="npsum_h", bufs=2, space="PSUM") as ps_h,
                tc.tile_pool(name="npsum_m", bufs=1, space="PSUM") as ps_m,
                tc.tile_pool(name="nsb", bufs=3) as nsb,
            ):
                bgp = ps_m.tile([P, OUT], fp32, tag="misc")
                nc.tensor.transpose(out=bgp[:], in_=bgs[:].to_broadcast([OUT, P]),
                                    identity=identf[:OUT, :OUT])
                bgmat = cst.tile([P, OUT], fp32)
                nc.vector.tensor_copy(bgmat[:], bgp[:])

                for t in range(TILES):
                    xTs = nsb.tile([P, KC * P], b16, tag="xTs")
                    nc.sync.dma_start(
                        out=xTs[:], in_=xT[:, t * KC * P:(t + 1) * KC * P])
                    hTp = ps_h.tile([EMB, P], fp32, tag="hT")
                    for k in range(KC):
                        nc.tensor.matmul(out=hTp[:],
                                         lhsT=wdTs[:, k * EMB:(k + 1) * EMB],
                                         rhs=xTs[:, k * P:(k + 1) * P],
                                         start=(k == 0), stop=(k == KC - 1))
                    u = nsb.tile([EMB, P], fp32, tag="u")
                    nc.scalar.activation(u[:], hTp[:],
                                         mybir.ActivationFunctionType.Identity,
                                         bias=bds[:, :1])
                    hT = nsb.tile([EMB, P], b16, tag="hT_sb")
                    nc.vector.scalar_tensor_tensor(
                        out=hT[:], in0=u[:], scalar=0.01, in1=u[:],
                        op0=mybir.AluOpType.mult, op1=mybir.AluOpType.max)
                    gTp = ps_m.tile([OUT, P], fp32, tag="misc")
                    nc.tensor.matmul(out=gTp[:], lhsT=wgTs[:], rhs=hT[:],
                                     start=True, stop=True)
                    stg = nsb.tile([OUT, P], b16, tag="stg")
                    nc.vector.tensor_copy(stg[:], gTp[:])
                    app = ps_m.tile([2, P], fp32, tag="app")
                    nc.tensor.matmul(out=app[:], lhsT=atts[:], rhs=stg[:],
                                     start=True, stop=True)
                    # transpose gT -> table g block
                    ttp = ps_m.tile([P, OUT], b16, tag="ttp")
                    nc.tensor.transpose(out=ttp[:], in_=stg[:],
                                        identity=identb[:OUT, :OUT])
                    tabs = nsb.tile([P, OUT], b16, tag="tabs")
                    nc.scalar.activation(tabs[:], ttp[:],
                                         mybir.ActivationFunctionType.Copy)
                    nc.sync.dma_start(
                        out=shard[t * P:(t + 1) * P, 0:OUT], in_=tabs[:])
                    # transpose [a_src; a_dst] -> [P, 2] f32
                    aps = nsb.tile([2, P], fp32, tag="aps")
                    nc.vector.tensor_copy(aps[:], app[:])
                    atp = ps_m.tile([P, 2], fp32, tag="atp")
                    nc.tensor.transpose(out=atp[:], in_=aps[:],
                                        identity=identf[:2, :2])
                    aTs = nsb.tile([P, 2], fp32, tag="aTs")
                    nc.vector.tensor_copy(aTs[:], atp[:])
                    nc.vector.tensor_copy(adst_all[:, t:t + 1], aTs[:, 1:2])
                    nc.sync.dma_start(
                        out=shard_f32[t * P:(t + 1) * P, A_SRC_F32:A_SRC_F32 + 2],
                        in_=aTs[:])

                # sentinel: a_src of the shard's last pad row := SENT_VAL
                nc.sync.dma_start(
                    out=shard_f32[NL_pad - 1:NL_pad, A_SRC_F32:A_SRC_F32 + 1],
                    in_=sentc[:])

            # ---------------- all-gather ----------------
            nc.gpsimd.collective_compute(
                "AllGather", mybir.AluOpType.bypass,
                replica_groups=[list(range(R))],
                ins=[shard.opt()], outs=[full.opt()],
            )

            # ---------------- edge phase ----------------
            halves = [full[0:NHALF, :], full[NHALF:NTAB, :]]
            qi = 0
            with tc.tile_pool(name="esb", bufs=3) as esb:
                cum = [0, 0]
                for t in range(TILES):
                    Dt = [int(D_lists[0][t]), int(D_lists[1][t])]
                    adst = adst_all[:, t:t + 1]
                    adst02 = esb.tile([P, 1], fp32, tag="adst02")
                    nc.scalar.activation(adst02[:], adst,
                                         mybir.ActivationFunctionType.Copy,
                                         scale=0.2)
                    DAB = Dt[0] + Dt[1]
                    gpads = []
                    T = esb.tile([P, DAB], fp32, tag="T")
                    off = 0
                    for h in range(2):
                        D = Dt[h]
                        oidx = esb.tile([P, 8 * D], i16, tag=f"oidx{h}")
                        nc.sync.dma_start(
                            out=oidx[:],
                            in_=offs_ext[h][:, 8 * cum[h]:8 * (cum[h] + D)])
                        gpad = esb.tile([P, D * W_ROW], b16, tag=f"gpad{h}")
                        for j in range(0, D, DCALL):
                            Dj = min(DCALL, D - j)
                            nc.gpsimd.dma_gather(
                                out_ap=gpad[:, j * W_ROW:(j + Dj) * W_ROW]
                                    .rearrange("p (d w) -> p d w", w=W_ROW),
                                in_ap=halves[h],
                                idxs_ap=oidx[:, 8 * j:8 * (j + Dj)],
                                num_idxs=P * Dj, num_idxs_reg=P * Dj,
                                elem_size=W_ROW,
                                queue_num=qi % 4,
                            )
                            qi += 1
                        gpads.append(gpad)
                        asrc = gpad[:].bitcast(fp32).rearrange(
                            "p (d w) -> p d w", w=W_ROW // 2)[:, :, A_SRC_F32]
                        t1 = esb.tile([P, D], fp32, tag=f"t1{h}")
                        nc.scalar.activation(t1[:], asrc,
                                             mybir.ActivationFunctionType.Exp,
                                             bias=adst, scale=1.0)
                        t2 = esb.tile([P, D], fp32, tag=f"t2{h}")
                        nc.scalar.activation(t2[:], asrc,
                                             mybir.ActivationFunctionType.Exp,
                                             bias=adst02[:, :1], scale=0.2)
                        nc.vector.tensor_tensor(out=T[:, off:off + D],
                                                in0=t1[:], in1=t2[:],
                                                op=mybir.AluOpType.max)
                        off += D
                    denom = esb.tile([P, 1], fp32, tag="denom")
                    nc.vector.tensor_reduce(out=denom[:], in_=T[:],
                                            op=mybir.AluOpType.add,
                                            axis=mybir.AxisListType.X)
                    rden = esb.tile([P, 1], fp32, tag="rden")
                    nc.vector.reciprocal(rden[:], denom[:])
                    gsc = esb.tile([P, DAB * OUT], b16, tag="gsc")
                    off = 0
                    for h in range(2):
                        D = Dt[h]
                        rows = gpads[h][:].rearrange("p (d w) -> p d w", w=W_ROW)
                        nc.vector.scalar_tensor_tensor(
                            out=gsc[:, off * OUT:(off + D) * OUT]
                                .rearrange("p (d c) -> p d c", c=OUT),
                            in0=rows[:, :, 0:OUT], scalar=rden[:, :1],
                            in1=T[:, off:off + D].to_broadcast([P, D, OUT]),
                            op0=mybir.AluOpType.mult, op1=mybir.AluOpType.mult)
                        off += D
                    onum = esb.tile([P, OUT], fp32, tag="onum")
                    nc.vector.tensor_reduce(
                        out=onum[:],
                        in_=gsc[:].rearrange("p (d c) -> p c d", c=OUT),
                        op=mybir.AluOpType.add, axis=mybir.AxisListType.X)
                    outf = esb.tile([P, OUT], fp32, tag="outf")
                    nc.vector.tensor_tensor(out=outf[:], in0=onum[:],
                                            in1=bgmat[:],
                                            op=mybir.AluOpType.add)
                    nc.sync.dma_start(out=out[t * P:(t + 1) * P, :], in_=outf[:])
                    cum[0] += Dt[0]
                    cum[1] += Dt[1]
    nc.finalize()
    return nc


# --------------------------------------------------------------------------
# entry points
# --------------------------------------------------------------------------

def run(inputs, cfg=CFG_REAL, trace=False):
    from concourse.bass_utils import run_bass_kernel_spmd
    in_maps, orders, D_lists = _prepare(cfg, **inputs)
    nc = _build_graph(cfg, D_lists)
    res = run_bass_kernel_spmd(nc, in_maps, core_ids=list(range(R)),
                               trace=trace)
    out = _assemble(cfg, res.results, orders)
    return out, res


def kernel(**inputs):
    inputs = {k: np.asarray(v) for k, v in inputs.items()}
    out, _ = run(inputs, CFG_REAL, trace=False)
    return out

